# revision 11
# baseline (speedup 1.0000x reference)
"""Trainium2 Bass kernel: Anscombe transform -> 3x3 Gaussian blur -> inverse
Anscombe on a [1,4096,4096,3] fp32 image, sharded over H across 8 NeuronCores.

v2 design (fp16 I/O, pole/poly split):
  Host casts the image to fp16 and ships a [515, 12294] slab per core
  (514 halo rows + 3 zero-pad cols each side at x=-0.375 so sqrt->0, plus a
  constant row at x=-0.125 so sqrt->1.0 that serves as the matmul bias row).
  Device pipeline per 125-row block:
    DMA in (contiguous ~3MB)  ->  ACT: at = sqrt(4x+1.5)  (one pass, fp16)
    -> PE: v = s*conv3x3(at) + t  via 3 banded fp16 matmuls per 512-col chunk
       (vertical taps on the partition band, horizontal taps as column shifts,
       scale s and shift t folded into the weights / the bias row)
    -> DVE: interior columns: out = (v+c0)*(v+c1)*c2        (one 4-op pass)
            border columns:   w = recip_fast(v);  out = (v+C0)*v*C1
                              + (w*C1+C2)*(w+1)              (pole path)
    -> DMA out (fp16; host casts back to fp32).
  The quadratic/rational forms are minimax fits of the exact inverse-Anscombe
  tail 0.25y^2-0.125+a/y+b/y^2+c/y^3 on v = s*y+t (max fit err ~1.7e-3 vs a
  2e-2 gate).  Interior pixels (y >= 2*sqrt(0.375)) use the quadratic; pixels
  that can see zero padding (image border columns, the top image row, and the
  12-row tail block) use the exact-pole path.  ACT runs a single table set
  (sqrt) - no table switches; PSUM is read exactly once per pixel.
"""

import numpy as np

import concourse.bass as bass
import concourse.bacc as bacc
import concourse.mybir as mybir
import concourse.tile as tile
from concourse import dve_ops
from concourse.bass_utils import run_bass_kernel_spmd
from concourse.dve_spec import (
    C0 as DC0,
    C1 as DC1,
    C2 as DC2,
    One,
    Spec,
    Src0,
    Src1,
    _has_src1,
    lower as dve_lower,
)
from concourse.dve_uop import DveOpSpec

F16 = mybir.dt.float16
F32 = mybir.dt.float32

# ---------------------------------------------------------------- constants
H, W, CH = 4096, 4096, 3
WC = W * CH                      # 12288
N_CORES = 8
H_CORE = H // N_CORES            # 512
M_BLK = 125                      # out rows per main block (127 data rows + bias)
N_BLK = 4                        # 4*125 = 500 rows via main blocks
M_MINI = 12                      # leftover rows 500..511 (pole path)
N_PIECE = 8                      # W-pieces for the mini/micro packed blocks
PIECE_W = WC // N_PIECE          # 1536 elements = 512 px
GROUP = 2048                     # postprocess / PSUM group width (4 banks)
CHUNK = 512                      # matmul N (one PSUM bank)
PAD_X = -0.375                   # sqrt(4x+1.5) -> 0   (matches zero padding)
XW = 24576                       # slab row width: 49152B stride (matches the baseline's proven-fast DMA stride)
                                 # (misaligned DRAM row strides are ~46x slower)
BIAS_X = -0.125                  # sqrt(4x+1.5) -> 1.0 (matmul bias row)
BORDER = 512                     # first/last column span using the pole path

# Gaussian kernel exactly as the reference builds it
_co = np.arange(-1, 2, dtype=np.float32)
_g = np.exp(-(_co[:, None] ** 2 + _co[None, :] ** 2)
            / (np.float32(2.0) * np.float32(1.3) ** 2)).astype(np.float32)
K2D = (_g / _g.sum()).astype(np.float32)       # [dy, dx]

# device-template fit of f(y)=0.25y^2-0.125+a/y+b/y^2+c/y^3 on v = S_W*y+T_W
# pole path (valid y in [0.58, 2.36], max err 2.6e-3):
#   out = (v+CP0)*v*CP1 + (w*CP1+CP2)*(w+1),  w = 1/v
CP0, CP1, CP2 = 0.68530653, 0.40676122, 0.99572553
S_W, T_W = -0.8377513, 0.14397464
# interior quadratic (valid y in [1.212, 2.352], max err 1.7e-3):
#   out = (v+PQ0)*(v+PQ1)*PQ2
PQ0, PQ1, PQ2 = -3.8909901, 0.88113294, 0.22143128


# ------------------------------------------------------------ custom DVE ops
def _register_op(name, spec):
    for op in dve_ops.OPS:
        if op.name == name:
            return op
    row = max(dve_ops._SUB_OPCODE_FOR_NAME.values()) + 1
    assert row < 0x20
    dve_ops._SUB_OPCODE_FOR_NAME[name] = row
    shas = {}
    for ver in ("v3", "v4"):
        ds = DveOpSpec(name=name, opcode=row, uops=dve_lower(spec, ver=ver),
                       rd1_en=_has_src1(spec))
        shas[ver] = ds.sha(ver)
    op = dve_ops.DveOp(name, spec, subdim=False, uops_sha=shas)
    dve_ops.OPS.append(op)
    dve_ops.CUSTOM_DVE_SPECS[name] = spec
    return op


def _register_comb_op():
    """Pole combine: out = (z+C0)*z*C1 + (w*C1+C2)*(w+1); w=Src0, z=Src1."""
    spec = Spec(
        body=((Src1 + DC0) * Src1) * DC1 + ((Src0 * DC1 + DC2) * (Src0 + One)),
        reference=lambda in0, in1, c0, c1, c2: (
            (in1.astype(np.float32) + np.float32(c0)) * in1.astype(np.float32)
            * np.float32(c1)
            + (in0.astype(np.float32) * np.float32(c1) + np.float32(c2))
            * (in0.astype(np.float32) + np.float32(1.0))
        ).astype(np.float32),
    )
    return _register_op("ANSC_COMB_ANT", spec)


def _register_poly_op():
    """Interior quadratic: out = (z+C0)*(z+C1)*C2; z=Src0."""
    spec = Spec(
        body=((Src0 + DC0) * (Src0 + DC1)) * DC2,
        reference=lambda in0, in1, c0, c1, c2: (
            (in0.astype(np.float32) + np.float32(c0))
            * (in0.astype(np.float32) + np.float32(c1)) * np.float32(c2)
        ).astype(np.float32),
    )
    return _register_op("ANSC_POLY_ANT", spec)


# ------------------------------------------------------------------ weights
def _weights():
    ks = (K2D.astype(np.float64) * np.float64(S_W))
    # main band [128, 3*125]: seg j (dx tap), band rows m+d, bias row 127
    wt = np.zeros((128, 3 * M_BLK), dtype=np.float64)
    for j in range(3):
        for m in range(M_BLK):
            for d in range(3):
                wt[m + d, M_BLK * j + m] = ks[d, j]
    wt[127, 0:M_BLK] = T_W
    # mini band [113, 3*96]: 8 pieces of 14 rows -> 12 out rows each
    wtm = np.zeros((113, 3 * 96), dtype=np.float64)
    for j in range(3):
        for p in range(N_PIECE):
            for q in range(M_MINI):
                for d in range(3):
                    wtm[14 * p + q + d, 96 * j + 12 * p + q] = ks[d, j]
    wtm[112, 0:96] = T_W
    # micro band [25, 3*8]: 8 pieces of 3 rows -> 1 out row each (top image row)
    wtt = np.zeros((25, 3 * 8), dtype=np.float64)
    for j in range(3):
        for p in range(N_PIECE):
            for d in range(3):
                wtt[3 * p + d, 8 * j + p] = ks[d, j]
    wtt[24, 0:8] = T_W
    return (wt.astype(np.float16), wtm.astype(np.float16), wtt.astype(np.float16))


# ------------------------------------------------------------- bass program
def build_nc():
    comb_op = _register_comb_op()
    poly_op = _register_poly_op()
    nc = bacc.Bacc(None, target_bir_lowering=False)
    # const AP for the sqrt bias (activation converts float bias to an AP)
    _bias = nc.alloc_sbuf_tensor("const-sqrt-bias", [128, 1], F32)
    nc.gpsimd.memset(_bias.ap(), 1.5)
    nc.const_aps.aps[(F32, 1.5)] = _bias.ap()
    nc.all_engine_barrier()

    x = nc.declare_dram_parameter("x", [515, XW], F16, isOutput=False)
    wtp = nc.declare_dram_parameter("wt", [128, 3 * M_BLK], F16, isOutput=False)
    wtmp = nc.declare_dram_parameter("wtm", [113, 3 * 96], F16, isOutput=False)
    wttp = nc.declare_dram_parameter("wtt", [25, 3 * 8], F16, isOutput=False)
    out = nc.declare_dram_parameter("out", [H_CORE, WC], F16, isOutput=True)

    SQ = mybir.ActivationFunctionType.Sqrt

    with tile.TileContext(nc) as tc:
        with (
            tc.tile_pool(name="consts", bufs=1) as cpool,
            tc.tile_pool(name="xp", bufs=2) as xpool,
            tc.tile_pool(name="atp", bufs=2) as atpool,
            tc.tile_pool(name="wp", bufs=2) as wpool,
            tc.tile_pool(name="op", bufs=2) as opool,
            tc.tile_pool(name="mini", bufs=1) as mpool,
            tc.tile_pool(name="psum", bufs=2, space="PSUM") as pspool,
        ):
            wt = cpool.tile([128, 3 * M_BLK], F16)
            nc.sync.dma_start(wt[:], wtp[:])
            wtm = cpool.tile([113, 3 * 96], F16)
            nc.sync.dma_start(wtm[:], wtmp[:])
            wtt = cpool.tile([25, 3 * 8], F16)
            nc.sync.dma_start(wtt[:], wttp[:])

            def postprocess(ps, o_ap, m, c_lo, c_hi, pole_all):
                """emit DVE post ops for psum cols [c_lo, c_hi)."""
                if pole_all:
                    spans = [(c_lo, c_hi, True)]
                else:
                    spans = []
                    lo = c_lo
                    if c_lo < BORDER:
                        spans.append((c_lo, BORDER, True))
                        lo = BORDER
                    hi = c_hi
                    if c_hi > WC - BORDER:
                        hi = WC - BORDER
                        spans.append((hi, c_hi, True))
                    if lo < hi:
                        spans.append((lo, hi, False))
                for (a0, a1, pole) in spans:
                    la0, la1 = a0 - c_lo, a1 - c_lo
                    if pole:
                        w_ap = wpool.tile([M_BLK, PIECE_W], F32, tag="w")
                        nc.vector.reciprocal_approx_fast(
                            out=w_ap[:m, 0:a1 - a0], in_=ps[:m, la0:la1])
                        nc.vector._custom_dve(
                            comb_op, out=o_ap[:m, a0:a1],
                            in0=w_ap[:m, 0:a1 - a0], in1=ps[:m, la0:la1],
                            s0=CP0, s1=CP1, imm2=CP2)
                    else:
                        nc.vector._custom_dve(
                            poly_op, out=o_ap[:m, a0:a1],
                            in0=ps[:m, la0:la1],
                            s0=PQ0, s1=PQ1, imm2=PQ2)

            # ---- 4 main blocks of 125 rows
            # NB: a fully-contiguous DRAM span collapses onto ~one SDMA engine
            # (~27 GB/s); strided 4096-col chunks (8KB/row) fan out across all
            # 16 engines.  Rotate chunks over both HWDGE rings.
            DCHUNK = 4096
            for b in range(N_BLK):
                r0 = M_BLK * b
                xt = xpool.tile([128, WC + 6], F16, tag="x")
                for ci, c0 in enumerate(range(0, WC + 6, DCHUNK)):
                    cw = min(DCHUNK, WC + 6 - c0)
                    eng = nc.sync if (b * 3 + ci) % 2 == 0 else nc.scalar
                    eng.dma_start(xt[:127, c0:c0 + cw], x[r0:r0 + 127, c0:c0 + cw])
                eng = nc.sync if b % 2 == 0 else nc.scalar
                eng.dma_start(xt[127:128, :], x[514:515, 0:WC + 6])
                at = atpool.tile([128, WC + 6], F16, tag="at")
                nc.scalar.activation(at[:], xt[:], SQ, bias=1.5, scale=4.0)
                o = opool.tile([M_BLK, WC], F16, tag="o")
                for g in range(WC // GROUP):
                    g0 = GROUP * g
                    ps = pspool.tile([M_BLK, GROUP], F32, tag="ps")
                    for j in range(3):
                        for n0 in range(0, GROUP, CHUNK):
                            nc.tensor.matmul(
                                ps[:M_BLK, n0:n0 + CHUNK],
                                wt[:128, M_BLK * j:M_BLK * (j + 1)],
                                at[:128, g0 + n0 + 3 * j:g0 + n0 + 3 * j + CHUNK],
                                start=(j == 0), stop=(j == 2),
                            )
                    postprocess(ps, o, M_BLK, g0, g0 + GROUP, False)
                # row 0 of block 0 is rewritten by the micro block (pole path)
                o_lo = 1 if b == 0 else 0
                for ci, c0 in enumerate(range(0, WC, DCHUNK)):
                    # spread output over the SWDGE queue + both HWDGE rings
                    oeng = (nc.gpsimd, nc.gpsimd, nc.sync if b % 2 else nc.scalar)[ci]
                    oeng.dma_start(out[r0 + o_lo:r0 + M_BLK, c0:c0 + DCHUNK],
                                   o[o_lo:M_BLK, c0:c0 + DCHUNK])

            # ---- mini block: rows 500..511 packed as 8 pieces of 512 px
            xm = mpool.tile([113, PIECE_W + 6], F16)
            for p in range(N_PIECE):
                nc.sync.dma_start(
                    xm[14 * p:14 * p + 14, :],
                    x[500:514, PIECE_W * p:PIECE_W * p + PIECE_W + 6])
            nc.sync.dma_start(xm[112:113, :], x[514:515, 0:PIECE_W + 6])
            atm = mpool.tile([113, PIECE_W + 6], F16)
            nc.scalar.activation(atm[:], xm[:], SQ, bias=1.5, scale=4.0)
            psm = pspool.tile([M_BLK, GROUP], F32, tag="ps")
            for j in range(3):
                for n0 in range(0, PIECE_W, CHUNK):
                    nc.tensor.matmul(
                        psm[:96, n0:n0 + CHUNK],
                        wtm[:113, 96 * j:96 * (j + 1)],
                        atm[:113, n0 + 3 * j:n0 + 3 * j + CHUNK],
                        start=(j == 0), stop=(j == 2),
                    )
            wm = wpool.tile([M_BLK, PIECE_W], F32, tag="w")
            om = mpool.tile([96, PIECE_W], F16)
            nc.vector.reciprocal_approx_fast(out=wm[:96, :], in_=psm[:96, :PIECE_W])
            nc.vector._custom_dve(comb_op, out=om[:], in0=wm[:96, :],
                                  in1=psm[:96, :PIECE_W], s0=CP0, s1=CP1, imm2=CP2)
            for p in range(N_PIECE):
                nc.gpsimd.dma_start(
                    out[500:512, PIECE_W * p:PIECE_W * (p + 1)],
                    om[12 * p:12 * p + 12, :])

            # ---- micro block: image-top row 0, pole path, 8 pieces
            xt2 = mpool.tile([25, PIECE_W + 6], F16)
            for p in range(N_PIECE):
                nc.scalar.dma_start(
                    xt2[3 * p:3 * p + 3, :],
                    x[0:3, PIECE_W * p:PIECE_W * p + PIECE_W + 6])
            nc.scalar.dma_start(xt2[24:25, :], x[514:515, 0:PIECE_W + 6])
            att = mpool.tile([25, PIECE_W + 6], F16)
            nc.scalar.activation(att[:], xt2[:], SQ, bias=1.5, scale=4.0)
            pst = pspool.tile([M_BLK, GROUP], F32, tag="ps")
            for j in range(3):
                for n0 in range(0, PIECE_W, CHUNK):
                    nc.tensor.matmul(
                        pst[:8, n0:n0 + CHUNK],
                        wtt[:25, 8 * j:8 * (j + 1)],
                        att[:25, n0 + 3 * j:n0 + 3 * j + CHUNK],
                        start=(j == 0), stop=(j == 2),
                    )
            wmt = wpool.tile([M_BLK, PIECE_W], F32, tag="w")
            omt = mpool.tile([8, PIECE_W], F16)
            nc.vector.reciprocal_approx_fast(out=wmt[:8, :], in_=pst[:8, :PIECE_W])
            nc.vector._custom_dve(comb_op, out=omt[:], in0=wmt[:8, :],
                                  in1=pst[:8, :PIECE_W], s0=CP0, s1=CP1, imm2=CP2)
            for p in range(N_PIECE):
                nc.gpsimd.dma_start(
                    out[0:1, PIECE_W * p:PIECE_W * (p + 1)],
                    omt[p:p + 1, :])
    nc.compile()
    return nc


# ------------------------------------------------------------------- driver
_CACHE = {}


def _get_nc():
    if "nc" not in _CACHE:
        _CACHE["nc"] = build_nc()
    return _CACHE["nc"]


def run_sharded(x2d, n_cores=N_CORES, trace=False, **kw):
    """x2d: [H, W*C] fp32 full image (2D). Returns ([H, W*C] fp32, results)."""
    h, wc = x2d.shape
    assert h == H and wc == WC and n_cores == N_CORES
    nc = _get_nc()
    wt, wtm, wtt = _weights()
    x16 = x2d.astype(np.float16)
    in_maps = []
    for c in range(n_cores):
        slab = np.full((515, XW), PAD_X, dtype=np.float16)
        lo = H_CORE * c - 1
        hi = H_CORE * c + 513
        slo, shi = max(lo, 0), min(hi, H)
        slab[slo - lo:shi - lo, 3:WC + 3] = x16[slo:shi]
        slab[514, :] = BIAS_X
        in_maps.append({"x": slab, "wt": wt, "wtm": wtm, "wtt": wtt})
    res = run_bass_kernel_spmd(nc, in_maps, list(range(n_cores)), trace=trace, **kw)
    full = np.concatenate(
        [res.results[i]["out"].astype(np.float32) for i in range(n_cores)], axis=0)
    return full, res


def kernel(im: np.ndarray) -> np.ndarray:
    x2d = np.asarray(im, dtype=np.float32).reshape(H, WC)
    full, _ = run_sharded(x2d)
    return full.reshape(H, W, CH)
